# revision 5
# baseline (speedup 1.0000x reference)
"""Trainium2 Bass kernel for nn_AttentionBlock (GroupNorm + 1x1-conv QKV +
full self-attention over N=HW=4096 + output projection + residual).

Distribution: data-parallel over batch B=8, one batch element per NeuronCore.

v2: all matmul operands in bf16 (fp32 matmuls stream at ~2 cyc/col on TRN2
hardware -- "fp32_mode=HIGH/LOW" dual pass -- so bf16 doubles PE throughput).
The loop processes queries in 4 blocks of 1024 so exp runs as a single
[128,1024] ACT instruction per j-tile, with double-buffered S tiles in PSUM:

  per (block, j):  S^T = K_j^T Q_blk   (PE, 2x 512-col matmuls, bf16)
                   P^T = exp(S^T)      (ACT, PSUM->SBUF bf16)
                   O  += V_j^T P^T     (PE, 2x 512-col, PSUM accum over j)
                   acc += P^T          (DVE, bf16 pair tree)

Softmax denominators: bf16 pair-tree accumulation on DVE, cross-partition
sum via gpsimd partition_all_reduce (idle Pool engine), reciprocal on a
DMA-scattered [128,8] layout, broadcast back via gpsimd partition_broadcast.
Tail uses proj/normalize commutation: O is normalized per-query BEFORE the
projection (they commute since the denominator is a per-query scalar), so
out = (w_proj^T (O * recip)) + (x + b_eff) with the residual pre-folded.

Bias algebra: b_k folded into the K PSUM->SBUF copies (ACT bias); b_q into
the Q copies (DVE); b_v folded into b_eff = b_proj + w_proj @ b_v (host,
exact). The attention scale C^-0.5 is folded into w_q/b_q on the host.
No max-subtraction in softmax: logits are ~N(0,1) so fp32 exp is safe.
"""

import numpy as np

B, C, H, W = 8, 128, 64, 64
HW = H * W                      # 4096
GROUPS = 8
GSIZE = C // GROUPS             # 16
EPS = 1e-5
NJ = HW // 128                  # 32 j-tiles
QW = 1024                       # queries per block
NQT = HW // QW                  # 4 blocks
SCALE = float(C) ** -0.5

_CACHE = {}


def _build():
    from contextlib import ExitStack

    import concourse.bacc as bacc
    import concourse.tile as tile
    from concourse import bass_isa, mybir

    f32 = mybir.dt.float32
    bf16 = mybir.dt.bfloat16
    AF = mybir.ActivationFunctionType

    nc = bacc.Bacc("TRN2", target_bir_lowering=False, debug=False)

    x_in = nc.dram_tensor("x", [C, HW], f32, kind="ExternalInput")
    gamma_in = nc.dram_tensor("gamma", [C, 1], f32, kind="ExternalInput")
    beta_in = nc.dram_tensor("beta", [C, 1], f32, kind="ExternalInput")
    bq_in = nc.dram_tensor("bq", [C, 1], f32, kind="ExternalInput")
    bk_in = nc.dram_tensor("bk", [C, 1], f32, kind="ExternalInput")
    beff_in = nc.dram_tensor("beff", [C, 1], f32, kind="ExternalInput")
    wq_in = nc.dram_tensor("wqT", [C, C], f32, kind="ExternalInput")
    wk_in = nc.dram_tensor("wkT", [C, C], f32, kind="ExternalInput")
    wv_in = nc.dram_tensor("wvT", [C, C], f32, kind="ExternalInput")
    wp_in = nc.dram_tensor("wpT", [C, C], f32, kind="ExternalInput")
    ig_in = nc.dram_tensor("ig", [C, GROUPS], f32, kind="ExternalInput")
    igt_in = nc.dram_tensor("igt", [GROUPS, C], f32, kind="ExternalInput")
    out_dram = nc.dram_tensor("out", [C, HW], f32, kind="ExternalOutput")

    with tile.TileContext(nc) as tc, ExitStack() as ctx:
        const = ctx.enter_context(tc.tile_pool(name="const", bufs=1))
        big = ctx.enter_context(tc.tile_pool(name="big", bufs=1))
        stats = ctx.enter_context(tc.tile_pool(name="stats", bufs=1))
        ptpool = ctx.enter_context(tc.tile_pool(name="pt", bufs=6))
        tmpool = ctx.enter_context(tc.tile_pool(name="tmp", bufs=2))
        denpool = ctx.enter_context(tc.tile_pool(name="den", bufs=2))
        rrpool = ctx.enter_context(tc.tile_pool(name="rr", bufs=2))
        onrmp = ctx.enter_context(tc.tile_pool(name="onrm", bufs=3))
        ostg = ctx.enter_context(tc.tile_pool(name="ostg", bufs=3))
        ps = ctx.enter_context(tc.tile_pool(name="ps", bufs=1, space="PSUM"))

        # ---------------- load x (split over two DMA queues), consts ------
        NCH = 4
        CHW = HW // NCH  # 1024
        x_sb = big.tile([C, HW], f32, tag="x")
        for ch in range(NCH):
            sl = slice(ch * CHW, (ch + 1) * CHW)
            eng = nc.sync if ch % 2 == 0 else nc.scalar
            eng.dma_start(x_sb[:, sl], x_in[:, sl])

        def cload(t_in, shape, tag):
            t = const.tile(shape, f32, tag=tag)
            nc.sync.dma_start(t[:], t_in[:])
            return t

        gamma = cload(gamma_in, [C, 1], "c_gamma")
        beta = cload(beta_in, [C, 1], "c_beta")
        bq = cload(bq_in, [C, 1], "c_bq")
        bk = cload(bk_in, [C, 1], "c_bk")
        beff = cload(beff_in, [C, 1], "c_beff")
        ig = cload(ig_in, [C, GROUPS], "c_ig")
        igt = cload(igt_in, [GROUPS, C], "c_igt")
        wq_f = cload(wq_in, [C, C], "c_wq_f")
        wk_f = cload(wk_in, [C, C], "c_wk_f")
        wv_f = cload(wv_in, [C, C], "c_wv_f")
        wp_f = cload(wp_in, [C, C], "c_wp_f")

        with nc.allow_low_precision(reason="bf16 weights: rel tol is 2e-2"):
            wq_b = const.tile([C, C], bf16)
            nc.vector.tensor_copy(wq_b[:], wq_f[:])
            wk_b = const.tile([C, C], bf16)
            nc.vector.tensor_copy(wk_b[:], wk_f[:])
            wv_b = const.tile([C, C], bf16)
            nc.vector.tensor_copy(wv_b[:], wv_f[:])
            wp_b = const.tile([C, C], bf16)
            nc.vector.tensor_copy(wp_b[:], wp_f[:])

        eps_t = const.tile([GROUPS, 1], f32)
        nc.vector.memset(eps_t[:], EPS)
        magic_t = const.tile([GROUPS, 1], mybir.dt.uint32)
        nc.vector.memset(magic_t[:], 0x5F3759DF)
        c15_t = const.tile([GROUPS, 1], f32)
        nc.vector.memset(c15_t[:], 1.5)

        # ---------------- groupnorm stats via bn_stats ----------------
        bnst = stats.tile([C, 8, 6], f32)
        for ch in range(8):
            sl = slice(ch * 512, (ch + 1) * 512)
            nc.vector.bn_stats(bnst[:, ch, :], x_sb[:, sl])
        mv = stats.tile([C, 2], f32)  # per-channel mean, var
        nc.vector.bn_aggr(mv[:], bnst[:])
        # warm the exp activation table before the loop
        warm = stats.tile([GROUPS, 1], f32)
        nc.scalar.activation(warm[:], eps_t[:], AF.Exp)

        # pack [mean, var + mean^2] -> group sums via indicator matmul
        msq = stats.tile([C, 2], f32)
        nc.vector.tensor_copy(msq[:, 0:1], mv[:, 0:1])
        nc.vector.tensor_mul(msq[:, 1:2], mv[:, 0:1], mv[:, 0:1])
        nc.vector.tensor_add(msq[:, 1:2], msq[:, 1:2], mv[:, 1:2])

        gs_ps = ps.tile([GROUPS, 2], f32, tag="pp0")
        nc.tensor.matmul(gs_ps[:], ig[:], msq[:], start=True, stop=True)
        gstats = stats.tile([GROUPS, 2], f32)
        nc.vector.tensor_scalar_mul(gstats[:], gs_ps[:], 1.0 / GSIZE)
        gmean = stats.tile([GROUPS, 1], f32)
        nc.vector.tensor_copy(gmean[:], gstats[:, 0:1])
        gmsq = stats.tile([GROUPS, 1], f32)
        nc.vector.tensor_mul(gmsq[:], gmean[:], gmean[:])
        gve = stats.tile([GROUPS, 1], f32)
        nc.vector.tensor_sub(gve[:], gstats[:, 1:2], gmsq[:])
        nc.vector.tensor_scalar(
            gve[:], gve[:], eps_t[:], None, mybir.AluOpType.add
        )
        # rstd = rsqrt(var+eps): quake guess + 2 Newton steps (DVE only)
        u32 = mybir.dt.uint32
        gu = stats.tile([GROUPS, 1], u32)
        nc.vector.tensor_scalar(
            gu[:], gve[:].bitcast(u32), 1, None,
            mybir.AluOpType.logical_shift_right,
        )
        nc.vector.tensor_sub(gu[:], magic_t[:], gu[:])
        gy = stats.tile([GROUPS, 1], f32)
        nc.vector.tensor_copy(gy[:], gu[:].bitcast(f32))
        gh = stats.tile([GROUPS, 1], f32)
        nc.vector.tensor_scalar_mul(gh[:], gve[:], 0.5)
        gt = stats.tile([GROUPS, 1], f32)
        for _ in range(2):
            nc.vector.tensor_mul(gt[:], gy[:], gy[:])
            nc.vector.tensor_mul(gt[:], gt[:], gh[:])
            nc.vector.tensor_sub(gt[:], c15_t[:], gt[:])
            nc.vector.tensor_mul(gy[:], gy[:], gt[:])
        gmr = stats.tile([GROUPS, 2], f32)
        nc.vector.tensor_copy(gmr[:, 1:2], gy[:])
        nc.vector.tensor_copy(gmr[:, 0:1], gmean[:])

        bc_ps = ps.tile([C, 2], f32, tag="pp1")
        nc.tensor.matmul(bc_ps[:], igt[:], gmr[:], start=True, stop=True)
        a_c = stats.tile([C, 1], f32)
        b_c = stats.tile([C, 1], f32)
        tmc = stats.tile([C, 1], f32)
        nc.vector.tensor_scalar_mul(a_c[:], gamma[:], bc_ps[:, 1:2])
        nc.vector.tensor_scalar_mul(tmc[:], a_c[:], bc_ps[:, 0:1])
        nc.vector.tensor_sub(b_c[:], beta[:], tmc[:])

        # ---------------- hn (bf16) and QKV ----------------
        hn = big.tile([C, HW], bf16, tag="hn")
        q_r = big.tile([C, HW], bf16, tag="q")
        k_r = big.tile([C, HW], bf16, tag="k")
        vt = big.tile([C, NJ, 128], bf16, tag="vt")
        acc = big.tile([C, HW], bf16, tag="acc")
        o_sb = big.tile([C, HW], bf16, tag="o")
        bp = big.tile([C, HW], bf16, tag="bp")

        lp = nc.allow_low_precision(reason="bf16 data path: rel tol is 2e-2")
        lp.__enter__()

        for ch in range(NCH):
            sl = slice(ch * CHW, (ch + 1) * CHW)
            nc.vector.tensor_scalar(
                hn[:, sl], x_sb[:, sl], a_c[:], b_c[:],
                mybir.AluOpType.mult, mybir.AluOpType.add,
            )

        def emit_k_round(r, tagp):  # [C, 1024] per round
            kp = ps.tile([C, QW], f32, tag=f"s{tagp}")
            for kk in range(2):
                off = r * QW + kk * 512
                nc.tensor.matmul(
                    kp[:, kk * 512:(kk + 1) * 512], wk_b[:],
                    hn[:, off:off + 512], start=True, stop=True,
                )
            nc.scalar.activation(
                k_r[:, r * QW:(r + 1) * QW], kp[:], AF.Identity, bias=bk[:]
            )

        def emit_q_round(r, tagp):
            qp = ps.tile([C, QW], f32, tag=f"s{tagp}")
            for kk in range(2):
                off = r * QW + kk * 512
                nc.tensor.matmul(
                    qp[:, kk * 512:(kk + 1) * 512], wq_b[:],
                    hn[:, off:off + 512], start=True, stop=True,
                )
            nc.vector.tensor_scalar(
                q_r[:, r * QW:(r + 1) * QW], qp[:], bq[:], None,
                mybir.AluOpType.add,
            )

        def emit_v_round(r, tagp):  # 4 n-tiles per round
            vp = ps.tile([C, 4, C], f32, tag=f"s{tagp}")
            for t in range(4):
                nt = r * 4 + t
                nc.tensor.matmul(
                    vp[:, t, :], hn[:, nt * 128:(nt + 1) * 128], wv_b[:],
                    start=True, stop=True,
                )
            nc.vector.tensor_copy(vt[:, r * 4:(r + 1) * 4, :], vp[:])

        # alternate PSUM slots so each round only waits for the round
        # two allocations back
        emit_k_round(0, 0)
        emit_q_round(0, 1)
        emit_k_round(1, 0)
        emit_k_round(2, 1)
        emit_k_round(3, 0)
        for r in range(8):
            emit_v_round(r, (r + 1) % 2)

        # residual+bias prefold: x_sb <- x + beff (x no longer needed raw)
        for ch in range(2):
            sl = slice(ch * 2048, (ch + 1) * 2048)
            nc.vector.tensor_scalar(
                x_sb[:, sl], x_sb[:, sl], beff[:], None, mybir.AluOpType.add
            )

        # ---------------- main attention loop ----------------
        for qt in range(NQT):
            qsl = slice(qt * QW, (qt + 1) * QW)
            op = ps.tile([C, QW], f32, tag="o_ps")
            pts = [None, None]
            for j in range(NJ):
                if j == 16 and qt < NQT - 1:
                    emit_q_round(qt + 1, j % 2)
                sp = ps.tile([C, QW], f32, tag=f"s{j % 2}")
                for kk in range(2):
                    qoff = qt * QW + kk * 512
                    nc.tensor.matmul(
                        sp[:, kk * 512:(kk + 1) * 512],
                        k_r[:, j * 128:(j + 1) * 128],
                        q_r[:, qoff:qoff + 512],
                        start=True, stop=True,
                    )
                pt = ptpool.tile([C, QW], bf16)
                nc.scalar.activation(pt[:], sp[:], AF.Exp)
                for kk in range(2):
                    sl = slice(kk * 512, (kk + 1) * 512)
                    nc.tensor.matmul(
                        op[:, sl], vt[:, j, :], pt[:, sl],
                        start=(j == 0), stop=(j == NJ - 1),
                    )
                pts[j % 2] = pt
                if j % 2 == 1:
                    if j == 1:
                        nc.vector.tensor_add(acc[:, qsl], pts[0][:], pts[1][:])
                    else:
                        tmp = tmpool.tile([C, QW], bf16)
                        nc.vector.tensor_add(tmp[:], pts[0][:], pts[1][:])
                        nc.vector.tensor_add(acc[:, qsl], acc[:, qsl], tmp[:])

            # ---- block epilogue (overlaps the next block's j-loop) ----
            nc.vector.tensor_copy(o_sb[:, qsl], op[:])
            den_q = denpool.tile([C, QW], f32, tag="den")
            nc.gpsimd.partition_all_reduce(
                den_q[:], acc[:, qsl], 128, bass_isa.ReduceOp.add
            )
            rs = rrpool.tile([C, 8], f32, tag="rs")
            nc.gpsimd.dma_start(rs[:], den_q[0:1, :])
            rc = rrpool.tile([C, 8], bf16, tag="rc")
            nc.vector.reciprocal(rc[:], rs[:])
            rrow = rrpool.tile([1, QW], bf16, tag="rrow")
            nc.gpsimd.dma_start(rrow[:], rc[:])
            nc.gpsimd.partition_broadcast(bp[:, qsl], rrow[:])
            for c2 in range(2):
                off = qt * QW + c2 * 512
                sl = slice(off, off + 512)
                onrm = onrmp.tile([C, 512], bf16)
                nc.vector.tensor_mul(onrm[:], o_sb[:, sl], bp[:, sl])
                pp = ps.tile([C, 512], f32, tag=f"pp{(qt * 2 + c2) % 2}")
                nc.tensor.matmul(pp[:], wp_b[:], onrm[:], start=True, stop=True)
                ost = ostg.tile([C, 512], f32)
                nc.vector.tensor_add(ost[:], pp[:], x_sb[:, sl])
                nc.sync.dma_start(out_dram[:, sl], ost[:])

        lp.__exit__(None, None, None)

    nc.compile()
    return nc


def _get_nc():
    if "nc" not in _CACHE:
        _CACHE["nc"] = _build()
    return _CACHE["nc"]


def _prep_inputs(x, gamma, beta, w_qkv, b_qkv, w_proj, b_proj):
    x = np.ascontiguousarray(x, dtype=np.float32)
    w_qkv = np.asarray(w_qkv, dtype=np.float32)
    b_qkv = np.asarray(b_qkv, dtype=np.float32)
    w_proj = np.asarray(w_proj, dtype=np.float32)
    b_proj = np.asarray(b_proj, dtype=np.float32)

    wq = w_qkv[0:C, :]
    wk = w_qkv[C:2 * C, :]
    wv = w_qkv[2 * C:3 * C, :]
    bqv = b_qkv[0:C]
    bkv = b_qkv[C:2 * C]
    bvv = b_qkv[2 * C:3 * C]

    wqT = np.ascontiguousarray((wq * SCALE).T)
    wkT = np.ascontiguousarray(wk.T)
    wvT = np.ascontiguousarray(wv.T)
    wpT = np.ascontiguousarray(w_proj.T)
    beff = (b_proj + w_proj @ bvv).astype(np.float32)

    ig = np.zeros((C, GROUPS), np.float32)
    ig[np.arange(C), np.arange(C) // GSIZE] = 1.0
    igt = np.ascontiguousarray(ig.T)

    common = {
        "gamma": np.asarray(gamma, np.float32).reshape(C, 1),
        "beta": np.asarray(beta, np.float32).reshape(C, 1),
        "bq": (bqv * SCALE).reshape(C, 1),
        "bk": bkv.reshape(C, 1),
        "beff": beff.reshape(C, 1),
        "wqT": wqT,
        "wkT": wkT,
        "wvT": wvT,
        "wpT": wpT,
        "ig": ig,
        "igt": igt,
    }
    in_maps = []
    for b in range(B):
        m = dict(common)
        m["x"] = np.ascontiguousarray(x[b].reshape(C, HW))
        in_maps.append(m)
    return in_maps


def kernel(x, gamma, beta, w_qkv, b_qkv, w_proj, b_proj):
    from concourse.bass_utils import run_bass_kernel_spmd

    nc = _get_nc()
    in_maps = _prep_inputs(x, gamma, beta, w_qkv, b_qkv, w_proj, b_proj)
    res = run_bass_kernel_spmd(nc, in_maps, list(range(B)))
    out = np.stack([res.results[b]["out"] for b in range(B)], axis=0)
    return out.reshape(B, C, H, W).astype(np.float32)


# revision 11
# speedup vs baseline: 1.0320x; 1.0320x over previous
"""Trainium2 Bass kernel for nn_AttentionBlock (GroupNorm + 1x1-conv QKV +
full self-attention over N=HW=4096 + output projection + residual).

Distribution: data-parallel over batch B=8, one batch element per NeuronCore.

v2: all matmul operands in bf16 (fp32 matmuls stream at ~2 cyc/col on TRN2
hardware -- "fp32_mode=HIGH/LOW" dual pass -- so bf16 doubles PE throughput).
The loop processes queries in 4 blocks of 1024 so exp runs as a single
[128,1024] ACT instruction per j-tile, with double-buffered S tiles in PSUM:

  per (block, j):  S^T = K_j^T Q_blk   (PE, 2x 512-col matmuls, bf16)
                   P^T = exp(S^T)      (ACT, PSUM->SBUF bf16)
                   O  += V_j^T P^T     (PE, 2x 512-col, PSUM accum over j)
                   acc += P^T          (DVE, bf16 pair tree)

Softmax denominators: bf16 pair-tree accumulation on DVE, cross-partition
sum via gpsimd partition_all_reduce (idle Pool engine), reciprocal on a
DMA-scattered [128,8] layout, broadcast back via gpsimd partition_broadcast.
Tail uses proj/normalize commutation: O is normalized per-query BEFORE the
projection (they commute since the denominator is a per-query scalar), so
out = (w_proj^T (O * recip)) + (x + b_eff) with the residual pre-folded.

Bias algebra: b_k folded into the K PSUM->SBUF copies (ACT bias); b_q into
the Q copies (DVE); b_v folded into b_eff = b_proj + w_proj @ b_v (host,
exact). The attention scale C^-0.5 is folded into w_q/b_q on the host.
No max-subtraction in softmax: logits are ~N(0,1) so fp32 exp is safe.
"""

import numpy as np

B, C, H, W = 8, 128, 64, 64
HW = H * W                      # 4096
GROUPS = 8
GSIZE = C // GROUPS             # 16
EPS = 1e-5
NJ = HW // 128                  # 32 j-tiles
QW = 1024                       # queries per block
NQT = HW // QW                  # 4 blocks
SCALE = float(C) ** -0.5

_CACHE = {}


def _build():
    from contextlib import ExitStack

    import concourse.bacc as bacc
    import concourse.tile as tile
    from concourse import bass_isa, mybir

    f32 = mybir.dt.float32
    bf16 = mybir.dt.bfloat16
    AF = mybir.ActivationFunctionType

    nc = bacc.Bacc("TRN2", target_bir_lowering=False, debug=False)

    x_in = nc.dram_tensor("x", [C, HW], f32, kind="ExternalInput")
    gamma_in = nc.dram_tensor("gamma", [C, 1], f32, kind="ExternalInput")
    beta_in = nc.dram_tensor("beta", [C, 1], f32, kind="ExternalInput")
    bq_in = nc.dram_tensor("bq", [C, 1], f32, kind="ExternalInput")
    bk_in = nc.dram_tensor("bk", [C, 1], f32, kind="ExternalInput")
    beff_in = nc.dram_tensor("beff", [C, 1], f32, kind="ExternalInput")
    wq_in = nc.dram_tensor("wqT", [C, C], f32, kind="ExternalInput")
    wk_in = nc.dram_tensor("wkT", [C, C], f32, kind="ExternalInput")
    wv_in = nc.dram_tensor("wvT", [C, C], f32, kind="ExternalInput")
    wp_in = nc.dram_tensor("wpT", [C, C], f32, kind="ExternalInput")
    ig_in = nc.dram_tensor("ig", [C, GROUPS], f32, kind="ExternalInput")
    igt_in = nc.dram_tensor("igt", [GROUPS, C], f32, kind="ExternalInput")
    out_dram = nc.dram_tensor("out", [C, HW], f32, kind="ExternalOutput")

    with tile.TileContext(nc) as tc, ExitStack() as ctx:
        const = ctx.enter_context(tc.tile_pool(name="const", bufs=1))
        big = ctx.enter_context(tc.tile_pool(name="big", bufs=1))
        stats = ctx.enter_context(tc.tile_pool(name="stats", bufs=1))
        ptpool = ctx.enter_context(tc.tile_pool(name="pt", bufs=12))
        tmpool = ctx.enter_context(tc.tile_pool(name="tmp", bufs=4))
        denpool = ctx.enter_context(tc.tile_pool(name="den", bufs=2))
        rrpool = ctx.enter_context(tc.tile_pool(name="rr", bufs=2))
        onrmp = ctx.enter_context(tc.tile_pool(name="onrm", bufs=3))
        ostg = ctx.enter_context(tc.tile_pool(name="ostg", bufs=3))
        ps = ctx.enter_context(tc.tile_pool(name="ps", bufs=1, space="PSUM"))

        # ---------------- load x (split over two DMA queues), consts ------
        NCH = 4
        CHW = HW // NCH  # 1024
        x_sb = big.tile([C, HW], f32, tag="x")
        for ch in range(NCH):
            sl = slice(ch * CHW, (ch + 1) * CHW)
            eng = nc.sync if ch % 2 == 0 else nc.scalar
            eng.dma_start(x_sb[:, sl], x_in[:, sl])

        def cload(t_in, shape, tag):
            t = const.tile(shape, f32, tag=tag)
            nc.sync.dma_start(t[:], t_in[:])
            return t

        gamma = cload(gamma_in, [C, 1], "c_gamma")
        beta = cload(beta_in, [C, 1], "c_beta")
        bq = cload(bq_in, [C, 1], "c_bq")
        bk = cload(bk_in, [C, 1], "c_bk")
        beff = cload(beff_in, [C, 1], "c_beff")
        ig = cload(ig_in, [C, GROUPS], "c_ig")
        igt = cload(igt_in, [GROUPS, C], "c_igt")
        wq_f = cload(wq_in, [C, C], "c_wq_f")
        wk_f = cload(wk_in, [C, C], "c_wk_f")
        wv_f = cload(wv_in, [C, C], "c_wv_f")
        wp_f = cload(wp_in, [C, C], "c_wp_f")

        with nc.allow_low_precision(reason="bf16 weights: rel tol is 2e-2"):
            wq_b = const.tile([C, C], bf16)
            nc.vector.tensor_copy(wq_b[:], wq_f[:])
            wk_b = const.tile([C, C], bf16)
            nc.vector.tensor_copy(wk_b[:], wk_f[:])
            wv_b = const.tile([C, C], bf16)
            nc.vector.tensor_copy(wv_b[:], wv_f[:])
            wp_b = const.tile([C, C], bf16)
            nc.vector.tensor_copy(wp_b[:], wp_f[:])

        ones_c = const.tile([C, 1], bf16)
        nc.vector.memset(ones_c[:], 1.0)
        ones_r = const.tile([1, C], bf16)
        nc.vector.memset(ones_r[:], 1.0)
        eps_t = const.tile([GROUPS, 1], f32)
        nc.vector.memset(eps_t[:], EPS)
        magic_t = const.tile([GROUPS, 1], mybir.dt.uint32)
        nc.vector.memset(magic_t[:], 0x5F3759DF)
        c15_t = const.tile([GROUPS, 1], f32)
        nc.vector.memset(c15_t[:], 1.5)

        # ---------------- groupnorm stats via bn_stats ----------------
        bnst = stats.tile([C, 8, 6], f32)
        for ch in range(8):
            sl = slice(ch * 512, (ch + 1) * 512)
            nc.vector.bn_stats(bnst[:, ch, :], x_sb[:, sl])
        mv = stats.tile([C, 2], f32)  # per-channel mean, var
        nc.vector.bn_aggr(mv[:], bnst[:])
        # warm the exp activation table before the loop
        warm = stats.tile([GROUPS, 1], f32)
        nc.scalar.activation(warm[:], eps_t[:], AF.Exp)

        # pack [mean, var + mean^2] -> group sums via indicator matmul
        msq = stats.tile([C, 2], f32)
        nc.vector.tensor_copy(msq[:, 0:1], mv[:, 0:1])
        nc.vector.tensor_mul(msq[:, 1:2], mv[:, 0:1], mv[:, 0:1])
        nc.vector.tensor_add(msq[:, 1:2], msq[:, 1:2], mv[:, 1:2])

        gs_ps = ps.tile([GROUPS, 2], f32, tag="pp0")
        nc.tensor.matmul(gs_ps[:], ig[:], msq[:], start=True, stop=True)
        gstats = stats.tile([GROUPS, 2], f32)
        nc.vector.tensor_scalar_mul(gstats[:], gs_ps[:], 1.0 / GSIZE)
        gmean = stats.tile([GROUPS, 1], f32)
        nc.vector.tensor_copy(gmean[:], gstats[:, 0:1])
        gmsq = stats.tile([GROUPS, 1], f32)
        nc.vector.tensor_mul(gmsq[:], gmean[:], gmean[:])
        gve = stats.tile([GROUPS, 1], f32)
        nc.vector.tensor_sub(gve[:], gstats[:, 1:2], gmsq[:])
        nc.vector.tensor_scalar(
            gve[:], gve[:], eps_t[:], None, mybir.AluOpType.add
        )
        # rstd = rsqrt(var+eps): quake guess + 2 Newton steps (DVE only)
        u32 = mybir.dt.uint32
        gu = stats.tile([GROUPS, 1], u32)
        nc.vector.tensor_scalar(
            gu[:], gve[:].bitcast(u32), 1, None,
            mybir.AluOpType.logical_shift_right,
        )
        nc.vector.tensor_sub(gu[:], magic_t[:], gu[:])
        gy = stats.tile([GROUPS, 1], f32)
        nc.vector.tensor_copy(gy[:], gu[:].bitcast(f32))
        gh = stats.tile([GROUPS, 1], f32)
        nc.vector.tensor_scalar_mul(gh[:], gve[:], 0.5)
        gt = stats.tile([GROUPS, 1], f32)
        for _ in range(2):
            nc.vector.tensor_mul(gt[:], gy[:], gy[:])
            nc.vector.tensor_mul(gt[:], gt[:], gh[:])
            nc.vector.tensor_sub(gt[:], c15_t[:], gt[:])
            nc.vector.tensor_mul(gy[:], gy[:], gt[:])
        gmr = stats.tile([GROUPS, 2], f32)
        nc.vector.tensor_copy(gmr[:, 1:2], gy[:])
        nc.vector.tensor_copy(gmr[:, 0:1], gmean[:])

        bc_ps = ps.tile([C, 2], f32, tag="pp1")
        nc.tensor.matmul(bc_ps[:], igt[:], gmr[:], start=True, stop=True)
        a_c = stats.tile([C, 1], f32)
        b_c = stats.tile([C, 1], f32)
        tmc = stats.tile([C, 1], f32)
        nc.vector.tensor_scalar_mul(a_c[:], gamma[:], bc_ps[:, 1:2])
        nc.vector.tensor_scalar_mul(tmc[:], a_c[:], bc_ps[:, 0:1])
        nc.vector.tensor_sub(b_c[:], beta[:], tmc[:])

        # ---------------- hn (bf16) and QKV ----------------
        hn = big.tile([C, HW], bf16, tag="hn")
        q_r = big.tile([C, HW], bf16, tag="q")
        k_r = big.tile([C, HW], bf16, tag="k")
        vt = big.tile([C, NJ, 128], bf16, tag="vt")
        acc = big.tile([C, HW], bf16, tag="acc")
        o_sb = big.tile([C, HW], bf16, tag="o")
        bp = big.tile([C, HW], bf16, tag="bp")

        lp = nc.allow_low_precision(reason="bf16 data path: rel tol is 2e-2")
        lp.__enter__()

        for ch in range(NCH):
            sl = slice(ch * CHW, (ch + 1) * CHW)
            nc.vector.tensor_scalar(
                hn[:, sl], x_sb[:, sl], a_c[:], b_c[:],
                mybir.AluOpType.mult, mybir.AluOpType.add,
            )

        def emit_k_round(r, tagp):  # [C, 1024] per round
            kp = ps.tile([C, QW], f32, tag=f"s{tagp}")
            for kk in range(2):
                off = r * QW + kk * 512
                nc.tensor.matmul(
                    kp[:, kk * 512:(kk + 1) * 512], wk_b[:],
                    hn[:, off:off + 512], start=True, stop=True,
                )
            nc.scalar.activation(
                k_r[:, r * QW:(r + 1) * QW], kp[:], AF.Identity, bias=bk[:]
            )

        def emit_q_round(r, tagp):
            qp = ps.tile([C, QW], f32, tag=f"s{tagp}")
            for kk in range(2):
                off = r * QW + kk * 512
                nc.tensor.matmul(
                    qp[:, kk * 512:(kk + 1) * 512], wq_b[:],
                    hn[:, off:off + 512], start=True, stop=True,
                )
            nc.vector.tensor_scalar(
                q_r[:, r * QW:(r + 1) * QW], qp[:], bq[:], None,
                mybir.AluOpType.add,
            )

        def emit_v_round(r, tagp):  # 4 n-tiles per round
            vp = ps.tile([C, 4, C], f32, tag=f"s{tagp}")
            for t in range(4):
                nt = r * 4 + t
                nc.tensor.matmul(
                    vp[:, t, :], hn[:, nt * 128:(nt + 1) * 128], wv_b[:],
                    start=True, stop=True,
                )
            nc.vector.tensor_copy(vt[:, r * 4:(r + 1) * 4, :], vp[:])

        # alternate PSUM slots so each round only waits for the round
        # two allocations back
        emit_k_round(0, 0)
        emit_q_round(0, 1)
        emit_k_round(1, 0)
        emit_k_round(2, 1)
        emit_k_round(3, 0)
        for r in range(8):
            emit_v_round(r, (r + 1) % 2)

        # residual+bias prefold: x_sb <- x + beff (x no longer needed raw)
        for ch in range(2):
            sl = slice(ch * 2048, (ch + 1) * 2048)
            nc.vector.tensor_scalar(
                x_sb[:, sl], x_sb[:, sl], beff[:], None, mybir.AluOpType.add
            )

        # ---------------- main attention loop ----------------
        for qt in range(NQT):
            qsl = slice(qt * QW, (qt + 1) * QW)
            op = ps.tile([C, QW], f32, tag="o_ps")
            pts = [None, None]
            for j in range(NJ):
                if j == 16 and qt < NQT - 1:
                    emit_q_round(qt + 1, j % 2)
                sp = ps.tile([C, QW], f32, tag=f"s{j % 2}")
                for kk in range(2):
                    qoff = qt * QW + kk * 512
                    nc.tensor.matmul(
                        sp[:, kk * 512:(kk + 1) * 512],
                        k_r[:, j * 128:(j + 1) * 128],
                        q_r[:, qoff:qoff + 512],
                        start=True, stop=True,
                    )
                pt = ptpool.tile([C, QW], bf16)
                nc.scalar.activation(pt[:], sp[:], AF.Exp)
                for kk in range(2):
                    sl = slice(kk * 512, (kk + 1) * 512)
                    nc.tensor.matmul(
                        op[:, sl], vt[:, j, :], pt[:, sl],
                        start=(j == 0), stop=(j == NJ - 1),
                    )
                pts[j % 2] = pt
                if j % 2 == 1:
                    if j == 1:
                        nc.vector.tensor_add(acc[:, qsl], pts[0][:], pts[1][:])
                    else:
                        tmp = tmpool.tile([C, QW], bf16)
                        nc.vector.tensor_add(tmp[:], pts[0][:], pts[1][:])
                        nc.vector.tensor_add(acc[:, qsl], acc[:, qsl], tmp[:])

            # ---- block epilogue (overlaps the next block's j-loop) ----
            nc.vector.tensor_copy(o_sb[:, qsl], op[:])
            last = qt == NQT - 1
            if not last:
                # mid blocks: denominator chain on the idle Pool engine
                den_q = denpool.tile([C, QW], f32, tag="den")
                nc.gpsimd.partition_all_reduce(
                    den_q[:], acc[:, qsl], 128, bass_isa.ReduceOp.add
                )
                rs = rrpool.tile([C, 8], f32, tag="rs")
                nc.gpsimd.dma_start(rs[:], den_q[0:1, :])
                rc = rrpool.tile([C, 8], bf16, tag="rc")
                nc.vector.reciprocal(rc[:], rs[:])
                rrow = rrpool.tile([1, QW], bf16, tag="rrow")
                nc.gpsimd.dma_start(rrow[:], rc[:])
                nc.gpsimd.partition_broadcast(bp[:, qsl], rrow[:])
            else:
                # final block: PSUM s-slots are free now -- PE rowsum and
                # broadcast matmuls avoid the slow serial gpsimd chain
                dps = ps.tile([1, QW], f32, tag="s0")
                for c2 in range(2):
                    nc.tensor.matmul(
                        dps[:, c2 * 512:(c2 + 1) * 512], ones_c[:],
                        acc[:, qt * QW + c2 * 512:qt * QW + (c2 + 1) * 512],
                        start=True, stop=True,
                    )
                den_row = rrpool.tile([1, QW], f32, tag="drow")
                nc.vector.tensor_copy(den_row[:], dps[:])
                rs = rrpool.tile([C, 8], f32, tag="rs")
                nc.sync.dma_start(rs[:], den_row[:])
                rc = rrpool.tile([C, 8], bf16, tag="rc")
                nc.vector.reciprocal(rc[:], rs[:])
                rrow = rrpool.tile([1, QW], bf16, tag="rrow")
                nc.sync.dma_start(rrow[:], rc[:])
                bps = ps.tile([C, QW], f32, tag="s1")
                for c2 in range(2):
                    nc.tensor.matmul(
                        bps[:, c2 * 512:(c2 + 1) * 512], ones_r[:],
                        rrow[:, c2 * 512:(c2 + 1) * 512],
                        start=True, stop=True,
                    )
            for c2 in range(2):
                off = qt * QW + c2 * 512
                sl = slice(off, off + 512)
                onrm = onrmp.tile([C, 512], bf16)
                if last:
                    nc.vector.tensor_mul(
                        onrm[:], o_sb[:, sl], bps[:, c2 * 512:(c2 + 1) * 512]
                    )
                else:
                    nc.vector.tensor_mul(onrm[:], o_sb[:, sl], bp[:, sl])
                pp = ps.tile([C, 512], f32, tag=f"pp{(qt * 2 + c2) % 2}")
                nc.tensor.matmul(pp[:], wp_b[:], onrm[:], start=True, stop=True)
                ost = ostg.tile([C, 512], f32)
                nc.vector.tensor_add(ost[:], pp[:], x_sb[:, sl])
                nc.sync.dma_start(out_dram[:, sl], ost[:])

        lp.__exit__(None, None, None)

    nc.compile()
    return nc


def _get_nc():
    if "nc" not in _CACHE:
        _CACHE["nc"] = _build()
    return _CACHE["nc"]


def _prep_inputs(x, gamma, beta, w_qkv, b_qkv, w_proj, b_proj):
    x = np.ascontiguousarray(x, dtype=np.float32)
    w_qkv = np.asarray(w_qkv, dtype=np.float32)
    b_qkv = np.asarray(b_qkv, dtype=np.float32)
    w_proj = np.asarray(w_proj, dtype=np.float32)
    b_proj = np.asarray(b_proj, dtype=np.float32)

    wq = w_qkv[0:C, :]
    wk = w_qkv[C:2 * C, :]
    wv = w_qkv[2 * C:3 * C, :]
    bqv = b_qkv[0:C]
    bkv = b_qkv[C:2 * C]
    bvv = b_qkv[2 * C:3 * C]

    wqT = np.ascontiguousarray((wq * SCALE).T)
    wkT = np.ascontiguousarray(wk.T)
    wvT = np.ascontiguousarray(wv.T)
    wpT = np.ascontiguousarray(w_proj.T)
    beff = (b_proj + w_proj @ bvv).astype(np.float32)

    ig = np.zeros((C, GROUPS), np.float32)
    ig[np.arange(C), np.arange(C) // GSIZE] = 1.0
    igt = np.ascontiguousarray(ig.T)

    common = {
        "gamma": np.asarray(gamma, np.float32).reshape(C, 1),
        "beta": np.asarray(beta, np.float32).reshape(C, 1),
        "bq": (bqv * SCALE).reshape(C, 1),
        "bk": bkv.reshape(C, 1),
        "beff": beff.reshape(C, 1),
        "wqT": wqT,
        "wkT": wkT,
        "wvT": wvT,
        "wpT": wpT,
        "ig": ig,
        "igt": igt,
    }
    in_maps = []
    for b in range(B):
        m = dict(common)
        m["x"] = np.ascontiguousarray(x[b].reshape(C, HW))
        in_maps.append(m)
    return in_maps


def kernel(x, gamma, beta, w_qkv, b_qkv, w_proj, b_proj):
    from concourse.bass_utils import run_bass_kernel_spmd

    nc = _get_nc()
    in_maps = _prep_inputs(x, gamma, beta, w_qkv, b_qkv, w_proj, b_proj)
    res = run_bass_kernel_spmd(nc, in_maps, list(range(B)))
    out = np.stack([res.results[b]["out"] for b in range(B)], axis=0)
    return out.reshape(B, C, H, W).astype(np.float32)
